# revision 50
# baseline (speedup 1.0000x reference)
"""Trainium2 Bass kernel for nn_DigitConvolutionalModel.

Model: x[B,784] -> conv3x3(valid, 28x28->26x26) -> flatten -> Linear(676,256)
       -> relu -> Linear(256,10).

The conv is linear, so it is folded into the first Linear on the host:
  h_pre = x @ W1eff + b1,  W1eff[784,256] = C @ W1.T  (C = conv as matrix)
leaving a plain 2-layer MLP for the device:
  out = relu(x @ W1eff + b1) @ W2.T + b2

Sharding: pure data parallelism over the batch dim across 8 NeuronCores
(8192 samples/core); weights replicated.

Numerics: x AND the layer-1 weights travel in fp8 E3M4 (4 mantissa
bits, 1 col/cycle on the PE same as bf16, so this halves HBM traffic at
zero PE cost). W1eff is pre-scaled by 16 so its values sit in E3M4's
normal range; the relu activation rescales by 1/16. W2 stays bf16,
accumulation fp32 in PSUM. Measured rel err 0.0173 vs the 0.02 gate.
(fp8 E4M3 DoubleRow would double the matmul rate but measures 0.034
rel err — fails the gate — and hi/lo-split corrections cost back the
entire 2x, so bf16-rate streaming is the accuracy-feasible optimum.)

DMA: the x tensor is laid out group-contiguous in HBM ([NGRP,P,cols])
so every group load is one fully sequential HBM read, and group loads
are split across BOTH hardware DGE rings (sync + scalar) — one ring
alone saturates near ~95-150 GB/s, below the ~250 GB/s the PE needs at
roofline. Group 0 loads k-chunk-by-chunk (65KB each) on sync so its
matmuls start almost immediately; g2/g3 ride scalar behind the (fp8,
halved) weight prologue so neither queues behind g0+g1 on sync. Tails
load in pairs on sync. Deep prefetch (10-buffer x pool, issued 2+
blocks ahead) keeps both rings busy without a mid-kernel power spike:
the earlier single-ring version ran the whole kernel at 259ns/matmul
under HAM power throttling; this version streams at the full-clock
216ns/matmul (512 cols @ 2.37GHz) with zero mid-kernel PE gaps.

Schedule: the 784-dim contraction is 6 full 128-row chunks plus a
16-row tail applied via 4 row-tiled matmuls packed into distinct 32-row
PE groups (they execute concurrently, ~4ns apart). Batch groups run in
blocks of 2 (4 layer-1 PSUM banks per block from a 6-bank ps1 pool).
Per block: both k-loops, then the tail pack (so relus start early),
then the previous block's deferred layer-2 flushes. Engine roles are
strict to avoid cross-block FIFO dependency cycles: scalar = relus
(+ the final group's bias via Identity activation), vector = bias adds,
gpsimd SWDGE = interior output stores (batched 4 groups per store),
final stores split across the scalar+sync rings. Warmup/filler matmuls
keep the PE busy through the startup DMA fill so the HAM clock gate
un-throttles early. (NOTE: do NOT zero-prime PSUM banks with 0-weight
matmuls — runs with primes consistently measured ~12us slower, locked
at the throttled 259ns/matmul rate.) The last block flushes all pending
layer-2s before g15's k-loop and splits the closing relu/bias work
across scalar and vector so the drain chain is ~2.5us.
"""

import sys

if "/opt/trn_rl_repo" not in sys.path:
    sys.path.insert(0, "/opt/trn_rl_repo")

import ml_dtypes
import numpy as np

f8 = ml_dtypes.float8_e3m4

B = 65536
NCORES = 8
BC = B // NCORES  # 8192 samples per core
P = 128
KM = 6            # full 128-row contraction chunks (rows 0..767)
KT = 16           # tail contraction rows (768..783)
NF1 = 256         # layer-1 output features (2 halves of 128)
NO = 10           # logits
NB = 512          # batch columns per matmul group (one PSUM bank, fp32)
NGRP = BC // NB   # 16 groups per core
NBLK = NGRP // 2  # 8 blocks of 2 groups
W1T0C = KM * NF1  # w1 main columns; then P tail columns (all fp8)
WSCALE = 16.0     # W1eff transport scale (undone in the relu activation)

_PROG = None


def _build_program():
    import concourse.tile as tile
    from concourse import bacc, mybir

    bf16 = mybir.dt.bfloat16
    f32 = mybir.dt.float32
    Relu = mybir.ActivationFunctionType.Relu
    Identity = mybir.ActivationFunctionType.Identity
    add = mybir.AluOpType.add
    amax = mybir.AluOpType.max

    nc = bacc.Bacc("TRN2", target_bir_lowering=False, debug=False,
                   num_devices=NCORES)
    f8 = mybir.dt.float8e3
    # group-contiguous layouts: each slice below is one fully sequential
    # HBM read (partition-major inside), which keeps the SDMA engines on
    # contiguous streams
    xb = nc.dram_tensor("xb", [NGRP, P, KM * NB], f8,
                        kind="ExternalInput").ap()
    xtl = nc.dram_tensor("xtl", [P, NBLK, NB], f8,
                         kind="ExternalInput").ap()
    # L1 weights (incl. the 16-row tail pack) travel as fp8 e3m4 scaled by
    # 16 (the relu activation rescales by 1/16); W2 stays bf16
    wp = nc.dram_tensor("wp", [P, W1T0C + P], f8, kind="ExternalInput").ap()
    wp2 = nc.dram_tensor("wp2", [P, 2 * NO], bf16, kind="ExternalInput").ap()
    b1 = nc.dram_tensor("b1", [P, 2], f32, kind="ExternalInput").ap()
    b2 = nc.dram_tensor("b2", [NO, 1], f32, kind="ExternalInput").ap()
    out = nc.dram_tensor("out", [NO, BC], f32, kind="ExternalOutput").ap()

    with tile.TileContext(nc) as tc:
        with (
            tc.tile_pool(name="singles", bufs=1) as singles,
            tc.tile_pool(name="xp", bufs=10) as xp,
            tc.tile_pool(name="xtp", bufs=8) as xtp,
            tc.tile_pool(name="hp", bufs=8) as hp,
            tc.tile_pool(name="op", bufs=5) as op,
            tc.tile_pool(name="ps1", bufs=6, space="PSUM") as ps1p,
            tc.tile_pool(name="ps2", bufs=2, space="PSUM") as ps2p,
        ):
            # warmup tile + matmuls: keep the PE busy from the first cycle
            wsb = singles.tile([P, P], bf16)
            nc.vector.memset(wsb, 0.0)
            wmp = ps1p.tile([32, P], f32, tag="ps1", name="warm")

            def filler(n):
                for i in range(n):
                    nc.tensor.matmul(wmp, wsb[:, :32], wsb,
                                     start=(i == 0), stop=(i == n - 1))

            filler(24)

            # ---- weights on the scalar HWDGE ring, sliced in the exact
            # order the opening matmuls consume them
            wpsb = singles.tile([P, W1T0C + P], f8)
            wpsb2 = singles.tile([P, 2 * NO], bf16)
            W1T0 = W1T0C

            def w1w(k, m):
                c = k * NF1 + m * P
                return wpsb[:, c:c + P]

            def w2w(m):
                c = m * NO
                return wpsb2[:, c:c + NO]

            nc.scalar.dma_start(out=wpsb[:, 0:512], in_=wp[:, 0:512])
            nc.scalar.dma_start(out=wpsb[:, 512:1024], in_=wp[:, 512:1024])
            nc.scalar.dma_start(out=wpsb[:, 1024:W1T0C + P],
                                in_=wp[:, 1024:W1T0C + P])
            nc.scalar.dma_start(out=wpsb2, in_=wp2)
            b1sb = singles.tile([P, 2], f32)
            nc.scalar.dma_start(out=b1sb, in_=b1)
            b2sb = singles.tile([NO, 1], f32)
            nc.scalar.dma_start(out=b2sb, in_=b2)

            # ---- x split across BOTH hardware DGE rings (sync + scalar):
            # one ring alone saturates at ~95-200 GB/s, below the ~266 GB/s
            # the PE needs at roofline. Group 0 loads k-chunk by k-chunk on
            # sync (65KB each, so its matmuls can start almost immediately),
            # then odd groups ride sync and even groups ride scalar (which
            # first carries the weight prologue).
            xg0sb = singles.tile([P, KM, NB], f8)
            for k in range(KM):
                nc.sync.dma_start(out=xg0sb[:, k],
                                  in_=xb[0][:, k * NB:(k + 1) * NB])
            xtiles = [None] * NGRP
            xttiles = [None] * NBLK

            def load_group(g):
                xg = xp.tile([P, KM, NB], f8, tag="x", name=f"x_{g}")
                # g2 AND g3 ride scalar so neither queues behind the
                # ~790KB of g0+g1 on sync during the cold start
                ring = nc.scalar if g in (2, 3) or g % 2 == 0 else nc.sync
                ring.dma_start(out=xg, in_=xb[g])
                xtiles[g] = xg

            def load_tail(b):
                # tails load in pairs (one dma_start covers two blocks) on
                # the sync ring, which has slack during the cold start;
                # xtp has enough bufs that these never block on a pool wait
                if xttiles[b] is not None:
                    return
                n = 2 if b + 1 < NBLK else 1
                xt = xtp.tile([P, n, NB], f8, tag="xt", name=f"xt_{b}")
                nc.sync.dma_start(out=xt, in_=xtl[:, b:b + n])
                for i in range(n):
                    xttiles[b + i] = xt[:, i]

            load_group(1)
            load_group(2)
            load_group(3)
            load_tail(0)

            osbs = [None] * (NGRP // 4)

            def layer2(hs, g):
                # output staging is grouped 4 groups per buffer so interior
                # stores are fewer, bigger SWDGE transfers
                sb, slot = g // 4, g % 4
                if osbs[sb] is None:
                    osbs[sb] = op.tile([NO, 4, NB], f32, tag="o",
                                       name=f"o_{sb}")
                osb = osbs[sb]
                ps2 = ps2p.tile([NO, NB], f32, tag="ps2", name=f"ps2_{g}")
                for m in range(2):
                    nc.tensor.matmul(ps2, w2w(m), hs[m],
                                     start=(m == 0), stop=(m == 1))
                # bias on vector, relus on scalar: an engine that runs both
                # forms a cross-block dependency cycle (bias waits on L2
                # matmuls which wait on relus queued behind the bias)
                nc.vector.tensor_scalar_add(osb[:, slot], ps2, b2sb)
                if g >= NGRP - 2:
                    # final groups: short store chains, one per HW ring
                    gs = slice(g * NB, (g + 1) * NB)
                    ring = nc.scalar if g == NGRP - 2 else nc.sync
                    ring.dma_start(out=out[:, gs], in_=osb[:, slot])
                elif g == NGRP - 3:
                    gs = slice(12 * NB, 14 * NB)
                    nc.gpsimd.dma_start(out=out[:, gs], in_=osb[:, 0:2])
                elif slot == 3:
                    # interior stores ride the software DGE on the idle
                    # gpsimd queue (one 80KB store per ~7us keeps up)
                    gs = slice(sb * 4 * NB, (sb + 1) * 4 * NB)
                    nc.gpsimd.dma_start(out=out[:, gs], in_=osb)

            def relu_pair(g, pss):
                # both relus on the scalar engine (its only job); the 1/16
                # undoes the fp8 weight transport scale
                hs = []
                for m in range(2):
                    h = hp.tile([P, NB], bf16, tag="h", name=f"h_{g}_{m}")
                    nc.scalar.activation(h, pss[(g, m)], Relu,
                                         bias=b1sb[:, m:m + 1],
                                         scale=1.0 / WSCALE)
                    hs.append(h)
                return hs

            def pack_pair(g, pss):
                # 16-row contraction tail for one group: 2 row-tiled
                # matmuls in distinct 32-row PE groups (concurrent)
                xt = xttiles[g // 2]
                base = 0 if g % 2 == 0 else 64
                for m in range(2):
                    rs = slice(base + 32 * m, base + 32 * m + KT)
                    nc.tensor.matmul(pss[(g, m)], wpsb[rs, W1T0:W1T0 + P],
                                     xt[rs], start=False, stop=True,
                                     tile_position=(base + 32 * m, 0))

            def pack_quad(blk, pss):
                g0, g1 = 2 * blk, 2 * blk + 1
                xt = xttiles[blk]
                for r, (g, m) in enumerate(
                        [(g0, 0), (g0, 1), (g1, 0), (g1, 1)]):
                    rs = slice(32 * r, 32 * r + KT)
                    nc.tensor.matmul(pss[(g, m)], wpsb[rs, W1T0:W1T0 + P],
                                     xt[rs], start=False, stop=True,
                                     tile_position=(32 * r, 0))

            pend = []
            # ================= block 0 (groups 0, 1) =================
            pss = {(g, m): ps1p.tile([P, NB], f32, tag="ps1",
                                     name=f"ps1_{g}_{m}")
                   for g in (0, 1) for m in range(2)}

            def mm0k(k):
                for m in range(2):
                    nc.tensor.matmul(pss[(0, m)], w1w(k, m), xg0sb[:, k],
                                     start=(k == 0), stop=False)

            # matmuls ordered by DMA arrival (weights wA=k0k1/wB=k2k3/wC
            # on scalar; x chunks k0..k5 on sync); fillers bridge the known
            # arrival bubbles so the HAM clock gate never sees a PE-idle
            # window
            mm0k(0)
            mm0k(1)                       # wA
            filler(6)
            mm0k(2)
            mm0k(3)                       # wB
            filler(6)
            mm0k(4)
            mm0k(5)                       # wC
            # per-group tail packs in block 0: g0's relus start ~2.5us
            # earlier, which un-gates block-2's PSUM bank recycling
            pack_pair(0, pss)
            pend.append((relu_pair(0, pss), 0))
            filler(4)
            # group 1 whole (N=512)
            for m in range(2):
                for k in range(KM):
                    nc.tensor.matmul(pss[(1, m)], w1w(k, m),
                                     xtiles[1][:, k],
                                     start=(k == 0), stop=False)
            load_group(4)
            load_group(5)
            load_tail(2)
            pack_pair(1, pss)
            pend.append((relu_pair(1, pss), 1))

            # ================= blocks 1..7 =================
            for blk in range(1, NBLK):
                g0, g1 = 2 * blk, 2 * blk + 1
                if 2 * blk + 4 < NGRP:
                    load_group(2 * blk + 4)
                    load_group(2 * blk + 5)
                    if blk + 2 < NBLK:
                        load_tail(blk + 2)

                pss = {(g, m): ps1p.tile([P, NB], f32, tag="ps1",
                                         name=f"ps1_{g}_{m}")
                       for g in (g0, g1) for m in range(2)}
                primed = set()

                def kloop(g):
                    for m in range(2):
                        for k in range(KM):
                            nc.tensor.matmul(
                                pss[(g, m)], w1w(k, m), xtiles[g][:, k],
                                start=(k == 0 and (g, m) not in primed),
                                stop=False)

                if blk < NBLK - 1:
                    # pack right after the k-loops so the relus start as
                    # early as possible; the deferred layer-2 flushes run
                    # after (their h inputs are then long ready)
                    kloop(g0)
                    kloop(g1)
                    pack_quad(blk, pss)
                    while pend:
                        layer2(*pend.pop(0))
                    pend.append((relu_pair(g0, pss), g0))
                    pend.append((relu_pair(g1, pss), g1))
                else:
                    # last block: per-group tails so the final store chain
                    # is short. Both pending blocks' layer-2s flush BEFORE
                    # g15's k-loop so their vector bias-adds overlap PE
                    # work instead of serializing at the end, and g15's
                    # second relu runs on the (idle) gpsimd engine so the
                    # closing chain is one relu + one L2 + one bias + store.
                    kloop(g0)
                    pack_pair(g0, pss)
                    while pend:
                        layer2(*pend.pop(0))
                    hs14 = relu_pair(g0, pss)
                    kloop(g1)
                    pack_pair(g1, pss)
                    # g15's relus issue first so they overlap g14's L2 on
                    # the PE; g15's bias rides scalar behind them
                    h0 = hp.tile([P, NB], bf16, tag="h", name=f"h_{g1}_0")
                    nc.scalar.activation(h0, pss[(g1, 0)], Relu,
                                         bias=b1sb[:, 0:1],
                                         scale=1.0 / WSCALE)
                    h1 = hp.tile([P, NB], bf16, tag="h", name=f"h_{g1}_1")
                    nc.scalar.activation(h1, pss[(g1, 1)], Relu,
                                         bias=b1sb[:, 1:2],
                                         scale=1.0 / WSCALE)
                    layer2(hs14, g0)
                    # g15's bias-add on the scalar engine (free once h0 is
                    # done) so it doesn't queue behind h1's relu on vector
                    blk15, j15 = g1 // 4, g1 % 4
                    osb15 = osbs[blk15]
                    ps2f = ps2p.tile([NO, NB], f32, tag="ps2",
                                     name=f"ps2_{g1}")
                    for m in range(2):
                        nc.tensor.matmul(ps2f, w2w(m), [h0, h1][m],
                                         start=(m == 0), stop=(m == 1))
                    nc.scalar.activation(osb15[:, j15], ps2f, Identity,
                                         bias=b2sb)
                    gs = slice(g1 * NB, (g1 + 1) * NB)
                    nc.sync.dma_start(out=out[:, gs], in_=osb15[:, j15])

    nc.compile()
    return nc


def _fold_weights(conv_w, W1):
    """W1eff[784,256] such that x @ W1eff == flatten(conv(x)) @ W1.T."""
    cw = conv_w.astype(np.float64)
    W1r = W1.astype(np.float64).reshape(NF1, 26, 26).transpose(1, 2, 0)
    W1eff = np.zeros((28, 28, NF1), np.float64)
    for dr in range(3):
        for dc in range(3):
            W1eff[dr:dr + 26, dc:dc + 26, :] += cw[dr, dc] * W1r
    return W1eff.reshape(784, NF1)


def _prep_inputs(x, conv_w, W1, b1, W2, b2):
    bf16 = ml_dtypes.bfloat16
    W1eff = _fold_weights(conv_w, W1) * WSCALE
    wpack = np.zeros((P, W1T0C + P), np.float64)
    wpack[:, :KM * NF1] = W1eff[:768].reshape(KM, P, NF1).transpose(
        1, 0, 2).reshape(P, KM * NF1)
    W1T0 = KM * NF1
    wpack[0:KT, W1T0:W1T0 + P] = W1eff[768:784, 0:128]
    wpack[32:32 + KT, W1T0:W1T0 + P] = W1eff[768:784, 128:256]
    wpack[64:64 + KT, W1T0:W1T0 + P] = W1eff[768:784, 0:128]
    wpack[96:96 + KT, W1T0:W1T0 + P] = W1eff[768:784, 128:256]
    wpack = wpack.astype(np.float32).astype(f8)
    wpack2 = np.ascontiguousarray(W2.T.astype(np.float64).reshape(
        2, P, NO).transpose(1, 0, 2).reshape(P, 2 * NO)).astype(bf16)
    b1p = np.ascontiguousarray(b1.astype(np.float32).reshape(2, P).T)
    b2p = b2.astype(np.float32).reshape(NO, 1)

    in_maps = []
    for c in range(NCORES):
        xcT = np.ascontiguousarray(
            x[c * BC:(c + 1) * BC].T).astype(f8)  # [784, BC]
        xmain = np.ascontiguousarray(
            xcT[:768].reshape(KM, P, NGRP, NB).transpose(2, 1, 0, 3)
        ).reshape(NGRP, P, KM * NB)
        xtail = np.zeros((P, NBLK, NB), f8)
        tl = xcT[768:784].reshape(KT, NBLK, 2, NB)
        xtail[0:KT] = tl[:, :, 0]
        xtail[32:32 + KT] = tl[:, :, 0]
        xtail[64:64 + KT] = tl[:, :, 1]
        xtail[96:96 + KT] = tl[:, :, 1]
        in_maps.append({
            "xb": xmain, "xtl": xtail,
            "wp": wpack, "wp2": wpack2, "b1": b1p, "b2": b2p,
        })
    return in_maps


def kernel(x, conv_w, W1, b1, W2, b2, _trace=False, _trace_kwargs=None):
    global _PROG
    from concourse import bass_utils

    x = np.asarray(x, dtype=np.float32)
    conv_w = np.asarray(conv_w, dtype=np.float32)
    W1 = np.asarray(W1, dtype=np.float32)
    b1 = np.asarray(b1, dtype=np.float32)
    W2 = np.asarray(W2, dtype=np.float32)
    b2 = np.asarray(b2, dtype=np.float32)
    assert x.shape == (B, 784), x.shape

    if _PROG is None:
        _PROG = _build_program()

    in_maps = _prep_inputs(x, conv_w, W1, b1, W2, b2)
    kwargs = dict(_trace_kwargs or {})
    res = bass_utils.run_bass_kernel_spmd(
        _PROG, in_maps, core_ids=list(range(NCORES)), trace=_trace, **kwargs)

    out = np.empty((B, NO), np.float32)
    for c in range(NCORES):
        out[c * BC:(c + 1) * BC] = res.results[c]["out"].T
    if _trace:
        return out, res
    return out



# revision 51
# speedup vs baseline: 1.0470x; 1.0470x over previous
"""Trainium2 Bass kernel for nn_DigitConvolutionalModel.

Model: x[B,784] -> conv3x3(valid, 28x28->26x26) -> flatten -> Linear(676,256)
       -> relu -> Linear(256,10).

The conv is linear, so it is folded into the first Linear on the host:
  h_pre = x @ W1eff + b1,  W1eff[784,256] = C @ W1.T  (C = conv as matrix)
leaving a plain 2-layer MLP for the device:
  out = relu(x @ W1eff + b1) @ W2.T + b2

Sharding: pure data parallelism over the batch dim across 8 NeuronCores
(8192 samples/core); weights replicated.

Numerics: x AND the layer-1 weights travel in fp8 E3M4 (4 mantissa
bits, 1 col/cycle on the PE same as bf16, so this halves HBM traffic at
zero PE cost). W1eff is pre-scaled by 16 so its values sit in E3M4's
normal range; the relu activation rescales by 1/16. W2 stays bf16,
accumulation fp32 in PSUM. Measured rel err 0.0173 vs the 0.02 gate.
(fp8 E4M3 DoubleRow would double the matmul rate but measures 0.034
rel err — fails the gate — and hi/lo-split corrections cost back the
entire 2x, so bf16-rate streaming is the accuracy-feasible optimum.)

DMA: the x tensor is laid out group-contiguous in HBM ([NGRP,P,cols])
so every group load is one fully sequential HBM read, and group loads
are split across BOTH hardware DGE rings (sync + scalar) — one ring
alone saturates near ~95-150 GB/s, below the ~250 GB/s the PE needs at
roofline. Group 0 loads k-chunk-by-chunk (65KB each) on sync so its
matmuls start almost immediately; g2/g3 ride scalar behind the (fp8,
halved) weight prologue so neither queues behind g0+g1 on sync. Tails
load in pairs on sync. Deep prefetch (10-buffer x pool, issued 2+
blocks ahead) keeps both rings busy without a mid-kernel power spike:
the earlier single-ring version ran the whole kernel at 259ns/matmul
under HAM power throttling; this version streams at the full-clock
216ns/matmul (512 cols @ 2.37GHz) with zero mid-kernel PE gaps.

Schedule: the 784-dim contraction is 6 full 128-row chunks plus a
16-row tail applied via 4 row-tiled matmuls packed into distinct 32-row
PE groups (they execute concurrently, ~4ns apart). Batch groups run in
blocks of 2 (4 layer-1 PSUM banks per block from a 6-bank ps1 pool).
Per block: both k-loops, then the tail pack (so relus start early),
then the previous block's deferred layer-2 flushes. Engine roles are
strict to avoid cross-block FIFO dependency cycles: scalar = relus
(+ the final group's bias via Identity activation), vector = bias adds,
gpsimd SWDGE = interior output stores (batched 4 groups per store),
final stores split across the scalar+sync rings. Warmup/filler matmuls
keep the PE busy through the startup DMA fill so the HAM clock gate
un-throttles early. (NOTE: do NOT zero-prime PSUM banks with 0-weight
matmuls — runs with primes consistently measured ~12us slower, locked
at the throttled 259ns/matmul rate.) The last block flushes all pending
layer-2s before g15's k-loop and splits the closing relu/bias work
across scalar and vector so the drain chain is ~2.5us.
"""

import sys

if "/opt/trn_rl_repo" not in sys.path:
    sys.path.insert(0, "/opt/trn_rl_repo")

import ml_dtypes
import numpy as np

f8 = ml_dtypes.float8_e3m4

B = 65536
NCORES = 8
BC = B // NCORES  # 8192 samples per core
P = 128
KM = 6            # full 128-row contraction chunks (rows 0..767)
KT = 16           # tail contraction rows (768..783)
NF1 = 256         # layer-1 output features (2 halves of 128)
NO = 10           # logits
NB = 512          # batch columns per matmul group (one PSUM bank, fp32)
NGRP = BC // NB   # 16 groups per core
NBLK = NGRP // 2  # 8 blocks of 2 groups
W1T0C = KM * NF1  # w1 main columns; then P tail columns (all fp8)
WSCALE = 16.0     # W1eff transport scale (undone in the relu activation)

_PROG = None


def _build_program():
    import concourse.tile as tile
    from concourse import bacc, mybir

    bf16 = mybir.dt.bfloat16
    f32 = mybir.dt.float32
    Relu = mybir.ActivationFunctionType.Relu
    Identity = mybir.ActivationFunctionType.Identity
    add = mybir.AluOpType.add
    amax = mybir.AluOpType.max

    nc = bacc.Bacc("TRN2", target_bir_lowering=False, debug=False,
                   num_devices=NCORES)
    f8 = mybir.dt.float8e3
    # group-contiguous layouts: each slice below is one fully sequential
    # HBM read (partition-major inside), which keeps the SDMA engines on
    # contiguous streams
    xb = nc.dram_tensor("xb", [NGRP, P, KM * NB], f8,
                        kind="ExternalInput").ap()
    xtl = nc.dram_tensor("xtl", [P, NBLK, NB], f8,
                         kind="ExternalInput").ap()
    # L1 weights (incl. the 16-row tail pack) travel as fp8 e3m4 scaled by
    # 16 (the relu activation rescales by 1/16); W2 stays bf16
    wp = nc.dram_tensor("wp", [P, W1T0C + P], f8, kind="ExternalInput").ap()
    wp2 = nc.dram_tensor("wp2", [P, 2 * NO], bf16, kind="ExternalInput").ap()
    b1 = nc.dram_tensor("b1", [P, 2], f32, kind="ExternalInput").ap()
    b2 = nc.dram_tensor("b2", [NO, 1], f32, kind="ExternalInput").ap()
    out = nc.dram_tensor("out", [NO, BC], f32, kind="ExternalOutput").ap()

    with tile.TileContext(nc) as tc:
        with (
            tc.tile_pool(name="singles", bufs=1) as singles,
            tc.tile_pool(name="xp", bufs=10) as xp,
            tc.tile_pool(name="xtp", bufs=8) as xtp,
            tc.tile_pool(name="hp", bufs=8) as hp,
            tc.tile_pool(name="op", bufs=5) as op,
            tc.tile_pool(name="ps1", bufs=6, space="PSUM") as ps1p,
            tc.tile_pool(name="ps2", bufs=2, space="PSUM") as ps2p,
        ):
            # warmup tile + matmuls: keep the PE busy from the first cycle
            wsb = singles.tile([P, P], bf16)
            nc.vector.memset(wsb, 0.0)
            wmp = ps1p.tile([32, P], f32, tag="ps1", name="warm")

            def filler(n):
                for i in range(n):
                    nc.tensor.matmul(wmp, wsb[:, :32], wsb,
                                     start=(i == 0), stop=(i == n - 1))

            filler(24)

            # ---- weights on the scalar HWDGE ring, sliced in the exact
            # order the opening matmuls consume them
            wpsb = singles.tile([P, W1T0C + P], f8)
            wpsb2 = singles.tile([P, 2 * NO], bf16)
            W1T0 = W1T0C

            def w1w(k, m):
                c = k * NF1 + m * P
                return wpsb[:, c:c + P]

            def w2w(m):
                c = m * NO
                return wpsb2[:, c:c + NO]

            nc.scalar.dma_start(out=wpsb[:, 0:512], in_=wp[:, 0:512])
            nc.scalar.dma_start(out=wpsb[:, 512:1024], in_=wp[:, 512:1024])
            nc.scalar.dma_start(out=wpsb[:, 1024:W1T0C + P],
                                in_=wp[:, 1024:W1T0C + P])
            nc.scalar.dma_start(out=wpsb2, in_=wp2)
            b1sb = singles.tile([P, 2], f32)
            nc.scalar.dma_start(out=b1sb, in_=b1)
            b2sb = singles.tile([NO, 1], f32)
            nc.scalar.dma_start(out=b2sb, in_=b2)

            # ---- x split across BOTH hardware DGE rings (sync + scalar):
            # one ring alone saturates at ~95-200 GB/s, below the ~266 GB/s
            # the PE needs at roofline. Group 0 loads k-chunk by k-chunk on
            # sync (65KB each, so its matmuls can start almost immediately),
            # then odd groups ride sync and even groups ride scalar (which
            # first carries the weight prologue).
            xg0sb = singles.tile([P, KM, NB], f8)
            for k in range(KM):
                nc.sync.dma_start(out=xg0sb[:, k],
                                  in_=xb[0][:, k * NB:(k + 1) * NB])
            xtiles = [None] * NGRP
            xttiles = [None] * NBLK

            def load_group(g):
                xg = xp.tile([P, KM, NB], f8, tag="x", name=f"x_{g}")
                # g2 AND g3 ride scalar so neither queues behind the
                # ~790KB of g0+g1 on sync during the cold start
                ring = nc.scalar if g in (2, 3) or g % 2 == 0 else nc.sync
                ring.dma_start(out=xg, in_=xb[g])
                xtiles[g] = xg

            def load_tail(b):
                # tails load in pairs (one dma_start covers two blocks) on
                # the sync ring, which has slack during the cold start;
                # xtp has enough bufs that these never block on a pool wait
                if xttiles[b] is not None:
                    return
                n = 2 if b + 1 < NBLK else 1
                xt = xtp.tile([P, n, NB], f8, tag="xt", name=f"xt_{b}")
                nc.sync.dma_start(out=xt, in_=xtl[:, b:b + n])
                for i in range(n):
                    xttiles[b + i] = xt[:, i]

            load_group(1)
            load_group(2)
            load_group(3)
            load_tail(0)

            osbs = [None] * (NGRP // 4)

            def layer2(hs, g):
                # output staging is grouped 4 groups per buffer so interior
                # stores are fewer, bigger SWDGE transfers
                sb, slot = g // 4, g % 4
                if osbs[sb] is None:
                    osbs[sb] = op.tile([NO, 4, NB], f32, tag="o",
                                       name=f"o_{sb}")
                osb = osbs[sb]
                ps2 = ps2p.tile([NO, NB], f32, tag="ps2", name=f"ps2_{g}")
                for m in range(2):
                    nc.tensor.matmul(ps2, w2w(m), hs[m],
                                     start=(m == 0), stop=(m == 1))
                # bias on vector, relus on scalar: an engine that runs both
                # forms a cross-block dependency cycle (bias waits on L2
                # matmuls which wait on relus queued behind the bias)
                nc.vector.tensor_scalar_add(osb[:, slot], ps2, b2sb)
                if g >= NGRP - 2:
                    # final groups: short store chains, one per HW ring
                    gs = slice(g * NB, (g + 1) * NB)
                    ring = nc.scalar if g == NGRP - 2 else nc.sync
                    ring.dma_start(out=out[:, gs], in_=osb[:, slot])
                elif g == NGRP - 3:
                    gs = slice(12 * NB, 14 * NB)
                    nc.gpsimd.dma_start(out=out[:, gs], in_=osb[:, 0:2])
                elif slot == 3:
                    # interior stores ride the software DGE on the idle
                    # gpsimd queue (one 80KB store per ~7us keeps up)
                    gs = slice(sb * 4 * NB, (sb + 1) * 4 * NB)
                    nc.gpsimd.dma_start(out=out[:, gs], in_=osb)

            def relu_pair(g, pss):
                # both relus on the scalar engine (its only job); the 1/16
                # undoes the fp8 weight transport scale
                hs = []
                for m in range(2):
                    h = hp.tile([P, NB], bf16, tag="h", name=f"h_{g}_{m}")
                    nc.scalar.activation(h, pss[(g, m)], Relu,
                                         bias=b1sb[:, m:m + 1],
                                         scale=1.0 / WSCALE)
                    hs.append(h)
                return hs

            def pack_pair(g, pss):
                # 16-row contraction tail for one group: 2 row-tiled
                # matmuls in distinct 32-row PE groups (concurrent)
                xt = xttiles[g // 2]
                base = 0 if g % 2 == 0 else 64
                for m in range(2):
                    rs = slice(base + 32 * m, base + 32 * m + KT)
                    nc.tensor.matmul(pss[(g, m)], wpsb[rs, W1T0:W1T0 + P],
                                     xt[rs], start=False, stop=True,
                                     tile_position=(base + 32 * m, 0))

            def pack_quad(blk, pss):
                g0, g1 = 2 * blk, 2 * blk + 1
                xt = xttiles[blk]
                for r, (g, m) in enumerate(
                        [(g0, 0), (g0, 1), (g1, 0), (g1, 1)]):
                    rs = slice(32 * r, 32 * r + KT)
                    nc.tensor.matmul(pss[(g, m)], wpsb[rs, W1T0:W1T0 + P],
                                     xt[rs], start=False, stop=True,
                                     tile_position=(32 * r, 0))

            pend = []
            # ================= block 0 (groups 0, 1) =================
            pss = {(g, m): ps1p.tile([P, NB], f32, tag="ps1",
                                     name=f"ps1_{g}_{m}")
                   for g in (0, 1) for m in range(2)}

            def mm0k(k):
                for m in range(2):
                    nc.tensor.matmul(pss[(0, m)], w1w(k, m), xg0sb[:, k],
                                     start=(k == 0), stop=False)

            # matmuls ordered by DMA arrival (weights wA=k0k1/wB=k2k3/wC
            # on scalar; x chunks k0..k5 on sync); fillers bridge the known
            # arrival bubbles so the HAM clock gate never sees a PE-idle
            # window
            mm0k(0)
            mm0k(1)                       # wA
            filler(6)
            mm0k(2)
            mm0k(3)                       # wB
            filler(6)
            mm0k(4)
            mm0k(5)                       # wC
            filler(4)
            # group 1 whole (N=512)
            for m in range(2):
                for k in range(KM):
                    nc.tensor.matmul(pss[(1, m)], w1w(k, m),
                                     xtiles[1][:, k],
                                     start=(k == 0), stop=False)
            load_group(4)
            load_group(5)
            load_tail(2)
            pack_quad(0, pss)
            pend.append((relu_pair(0, pss), 0))
            pend.append((relu_pair(1, pss), 1))

            # ================= blocks 1..7 =================
            for blk in range(1, NBLK):
                g0, g1 = 2 * blk, 2 * blk + 1
                if 2 * blk + 4 < NGRP:
                    load_group(2 * blk + 4)
                    load_group(2 * blk + 5)
                    if blk + 2 < NBLK:
                        load_tail(blk + 2)

                pss = {(g, m): ps1p.tile([P, NB], f32, tag="ps1",
                                         name=f"ps1_{g}_{m}")
                       for g in (g0, g1) for m in range(2)}
                primed = set()

                def kloop(g):
                    for m in range(2):
                        for k in range(KM):
                            nc.tensor.matmul(
                                pss[(g, m)], w1w(k, m), xtiles[g][:, k],
                                start=(k == 0 and (g, m) not in primed),
                                stop=False)

                if blk < NBLK - 1:
                    # pack right after the k-loops so the relus start as
                    # early as possible; the deferred layer-2 flushes run
                    # after (their h inputs are then long ready)
                    kloop(g0)
                    kloop(g1)
                    pack_quad(blk, pss)
                    while pend:
                        layer2(*pend.pop(0))
                    pend.append((relu_pair(g0, pss), g0))
                    pend.append((relu_pair(g1, pss), g1))
                else:
                    # last block: per-group tails so the final store chain
                    # is short. Both pending blocks' layer-2s flush BEFORE
                    # g15's k-loop so their vector bias-adds overlap PE
                    # work instead of serializing at the end, and g15's
                    # second relu runs on the (idle) gpsimd engine so the
                    # closing chain is one relu + one L2 + one bias + store.
                    kloop(g0)
                    pack_pair(g0, pss)
                    while pend:
                        layer2(*pend.pop(0))
                    hs14 = relu_pair(g0, pss)
                    kloop(g1)
                    pack_pair(g1, pss)
                    # g15's relus issue first so they overlap g14's L2 on
                    # the PE; g15's bias rides scalar behind them
                    h0 = hp.tile([P, NB], bf16, tag="h", name=f"h_{g1}_0")
                    nc.scalar.activation(h0, pss[(g1, 0)], Relu,
                                         bias=b1sb[:, 0:1],
                                         scale=1.0 / WSCALE)
                    h1 = hp.tile([P, NB], bf16, tag="h", name=f"h_{g1}_1")
                    nc.scalar.activation(h1, pss[(g1, 1)], Relu,
                                         bias=b1sb[:, 1:2],
                                         scale=1.0 / WSCALE)
                    layer2(hs14, g0)
                    # g15's bias-add on the scalar engine (free once h0 is
                    # done) so it doesn't queue behind h1's relu on vector
                    blk15, j15 = g1 // 4, g1 % 4
                    osb15 = osbs[blk15]
                    ps2f = ps2p.tile([NO, NB], f32, tag="ps2",
                                     name=f"ps2_{g1}")
                    for m in range(2):
                        nc.tensor.matmul(ps2f, w2w(m), [h0, h1][m],
                                         start=(m == 0), stop=(m == 1))
                    nc.scalar.activation(osb15[:, j15], ps2f, Identity,
                                         bias=b2sb)
                    gs = slice(g1 * NB, (g1 + 1) * NB)
                    nc.sync.dma_start(out=out[:, gs], in_=osb15[:, j15])

    nc.compile()
    return nc


def _fold_weights(conv_w, W1):
    """W1eff[784,256] such that x @ W1eff == flatten(conv(x)) @ W1.T."""
    cw = conv_w.astype(np.float64)
    W1r = W1.astype(np.float64).reshape(NF1, 26, 26).transpose(1, 2, 0)
    W1eff = np.zeros((28, 28, NF1), np.float64)
    for dr in range(3):
        for dc in range(3):
            W1eff[dr:dr + 26, dc:dc + 26, :] += cw[dr, dc] * W1r
    return W1eff.reshape(784, NF1)


def _prep_inputs(x, conv_w, W1, b1, W2, b2):
    bf16 = ml_dtypes.bfloat16
    W1eff = _fold_weights(conv_w, W1) * WSCALE
    wpack = np.zeros((P, W1T0C + P), np.float64)
    wpack[:, :KM * NF1] = W1eff[:768].reshape(KM, P, NF1).transpose(
        1, 0, 2).reshape(P, KM * NF1)
    W1T0 = KM * NF1
    wpack[0:KT, W1T0:W1T0 + P] = W1eff[768:784, 0:128]
    wpack[32:32 + KT, W1T0:W1T0 + P] = W1eff[768:784, 128:256]
    wpack[64:64 + KT, W1T0:W1T0 + P] = W1eff[768:784, 0:128]
    wpack[96:96 + KT, W1T0:W1T0 + P] = W1eff[768:784, 128:256]
    wpack = wpack.astype(np.float32).astype(f8)
    wpack2 = np.ascontiguousarray(W2.T.astype(np.float64).reshape(
        2, P, NO).transpose(1, 0, 2).reshape(P, 2 * NO)).astype(bf16)
    b1p = np.ascontiguousarray(b1.astype(np.float32).reshape(2, P).T)
    b2p = b2.astype(np.float32).reshape(NO, 1)

    in_maps = []
    for c in range(NCORES):
        xcT = np.ascontiguousarray(
            x[c * BC:(c + 1) * BC].T).astype(f8)  # [784, BC]
        xmain = np.ascontiguousarray(
            xcT[:768].reshape(KM, P, NGRP, NB).transpose(2, 1, 0, 3)
        ).reshape(NGRP, P, KM * NB)
        xtail = np.zeros((P, NBLK, NB), f8)
        tl = xcT[768:784].reshape(KT, NBLK, 2, NB)
        xtail[0:KT] = tl[:, :, 0]
        xtail[32:32 + KT] = tl[:, :, 0]
        xtail[64:64 + KT] = tl[:, :, 1]
        xtail[96:96 + KT] = tl[:, :, 1]
        in_maps.append({
            "xb": xmain, "xtl": xtail,
            "wp": wpack, "wp2": wpack2, "b1": b1p, "b2": b2p,
        })
    return in_maps


def kernel(x, conv_w, W1, b1, W2, b2, _trace=False, _trace_kwargs=None):
    global _PROG
    from concourse import bass_utils

    x = np.asarray(x, dtype=np.float32)
    conv_w = np.asarray(conv_w, dtype=np.float32)
    W1 = np.asarray(W1, dtype=np.float32)
    b1 = np.asarray(b1, dtype=np.float32)
    W2 = np.asarray(W2, dtype=np.float32)
    b2 = np.asarray(b2, dtype=np.float32)
    assert x.shape == (B, 784), x.shape

    if _PROG is None:
        _PROG = _build_program()

    in_maps = _prep_inputs(x, conv_w, W1, b1, W2, b2)
    kwargs = dict(_trace_kwargs or {})
    res = bass_utils.run_bass_kernel_spmd(
        _PROG, in_maps, core_ids=list(range(NCORES)), trace=_trace, **kwargs)

    out = np.empty((B, NO), np.float32)
    for c in range(NCORES):
        out[c * BC:(c + 1) * BC] = res.results[c]["out"].T
    if _trace:
        return out, res
    return out



# revision 55
# speedup vs baseline: 1.0565x; 1.0091x over previous
"""Trainium2 Bass kernel for nn_DigitConvolutionalModel.

Model: x[B,784] -> conv3x3(valid, 28x28->26x26) -> flatten -> Linear(676,256)
       -> relu -> Linear(256,10).

The conv is linear, so it is folded into the first Linear on the host:
  h_pre = x @ W1eff + b1,  W1eff[784,256] = C @ W1.T  (C = conv as matrix)
leaving a plain 2-layer MLP for the device:
  out = relu(x @ W1eff + b1) @ W2.T + b2

Sharding: pure data parallelism over the batch dim across 8 NeuronCores
(8192 samples/core); weights replicated.

Numerics: x AND the layer-1 weights travel in fp8 E3M4 (4 mantissa
bits, 1 col/cycle on the PE same as bf16, so this halves HBM traffic at
zero PE cost). W1eff is pre-scaled by 16 so its values sit in E3M4's
normal range; the relu activation rescales by 1/16. W2 stays bf16,
accumulation fp32 in PSUM. Measured rel err 0.0173 vs the 0.02 gate.
(fp8 E4M3 DoubleRow would double the matmul rate but measures 0.034
rel err — fails the gate — and hi/lo-split corrections cost back the
entire 2x, so bf16-rate streaming is the accuracy-feasible optimum.)

DMA: the x tensor is laid out group-contiguous in HBM ([NGRP,P,cols])
so every group load is one fully sequential HBM read, and group loads
are split across BOTH hardware DGE rings (sync + scalar) — one ring
alone saturates near ~95-150 GB/s, below the ~250 GB/s the PE needs at
roofline. Group 0 loads k-chunk-by-chunk (65KB each) on sync so its
matmuls start almost immediately; g2/g3 ride scalar behind the (fp8,
halved) weight prologue so neither queues behind g0+g1 on sync. Tails
load in pairs on sync. Deep prefetch (10-buffer x pool, issued 2+
blocks ahead) keeps both rings busy without a mid-kernel power spike:
the earlier single-ring version ran the whole kernel at 259ns/matmul
under HAM power throttling; this version streams at the full-clock
216ns/matmul (512 cols @ 2.37GHz) with zero mid-kernel PE gaps.

Schedule: the 784-dim contraction is 6 full 128-row chunks plus a
16-row tail applied via 4 row-tiled matmuls packed into distinct 32-row
PE groups (they execute concurrently, ~4ns apart). Batch groups run in
blocks of 2 (4 layer-1 PSUM banks per block from a 6-bank ps1 pool).
Per block: both k-loops, then the tail pack (so relus start early),
then the previous block's deferred layer-2 flushes. Engine roles are
strict to avoid cross-block FIFO dependency cycles: scalar = relus
(+ the final group's bias via Identity activation), vector = bias adds,
gpsimd SWDGE = interior output stores (batched 4 groups per store),
final stores split across the scalar+sync rings. Warmup/filler matmuls
keep the PE busy through the startup DMA fill so the HAM clock gate
un-throttles early. (NOTE: do NOT zero-prime PSUM banks with 0-weight
matmuls — runs with primes consistently measured ~12us slower, locked
at the throttled 259ns/matmul rate.) The last block flushes all pending
layer-2s before g15's k-loop and splits the closing relu/bias work
across scalar and vector so the drain chain is ~2.5us.
"""

import sys

if "/opt/trn_rl_repo" not in sys.path:
    sys.path.insert(0, "/opt/trn_rl_repo")

import ml_dtypes
import numpy as np

f8 = ml_dtypes.float8_e3m4

B = 65536
NCORES = 8
BC = B // NCORES  # 8192 samples per core
P = 128
KM = 6            # full 128-row contraction chunks (rows 0..767)
KT = 16           # tail contraction rows (768..783)
NF1 = 256         # layer-1 output features (2 halves of 128)
NO = 10           # logits
NB = 512          # batch columns per matmul group (one PSUM bank, fp32)
NGRP = BC // NB   # 16 groups per core
NBLK = NGRP // 2  # 8 blocks of 2 groups
W1T0C = KM * NF1  # w1 main columns; then P tail columns (all fp8)
WSCALE = 16.0     # W1eff transport scale (undone in the relu activation)

_PROG = None


def _build_program():
    import concourse.tile as tile
    from concourse import bacc, mybir

    bf16 = mybir.dt.bfloat16
    f32 = mybir.dt.float32
    Relu = mybir.ActivationFunctionType.Relu
    Identity = mybir.ActivationFunctionType.Identity
    add = mybir.AluOpType.add
    amax = mybir.AluOpType.max

    nc = bacc.Bacc("TRN2", target_bir_lowering=False, debug=False,
                   num_devices=NCORES)
    f8 = mybir.dt.float8e3
    # group-contiguous layouts: each slice below is one fully sequential
    # HBM read (partition-major inside), which keeps the SDMA engines on
    # contiguous streams
    xb = nc.dram_tensor("xb", [NGRP, P, KM * NB], f8,
                        kind="ExternalInput").ap()
    xtl = nc.dram_tensor("xtl", [P, NBLK, NB], f8,
                         kind="ExternalInput").ap()
    # L1 weights (incl. the 16-row tail pack) travel as fp8 e3m4 scaled by
    # 16 (the relu activation rescales by 1/16); W2 stays bf16
    wp = nc.dram_tensor("wp", [P, W1T0C + P], f8, kind="ExternalInput").ap()
    wp2 = nc.dram_tensor("wp2", [P, 2 * NO], bf16, kind="ExternalInput").ap()
    b1 = nc.dram_tensor("b1", [P, 2], f32, kind="ExternalInput").ap()
    b2 = nc.dram_tensor("b2", [NO, 1], f32, kind="ExternalInput").ap()
    out = nc.dram_tensor("out", [NO, BC], f32, kind="ExternalOutput").ap()

    with tile.TileContext(nc) as tc:
        with (
            tc.tile_pool(name="singles", bufs=1) as singles,
            tc.tile_pool(name="xp", bufs=10) as xp,
            tc.tile_pool(name="xtp", bufs=8) as xtp,
            tc.tile_pool(name="hp", bufs=8) as hp,
            tc.tile_pool(name="op", bufs=5) as op,
            tc.tile_pool(name="ps1", bufs=6, space="PSUM") as ps1p,
            tc.tile_pool(name="ps2", bufs=2, space="PSUM") as ps2p,
        ):
            # warmup tile + matmuls: keep the PE busy from the first cycle
            wsb = singles.tile([P, P], bf16)
            nc.vector.memset(wsb, 0.0)
            wmp = ps1p.tile([32, P], f32, tag="ps1", name="warm")

            def filler(n):
                for i in range(n):
                    nc.tensor.matmul(wmp, wsb[:, :32], wsb,
                                     start=(i == 0), stop=(i == n - 1))

            filler(24)

            # ---- weights on the scalar HWDGE ring, sliced in the exact
            # order the opening matmuls consume them
            wpsb = singles.tile([P, W1T0C + P], f8)
            wpsb2 = singles.tile([P, 2 * NO], bf16)
            W1T0 = W1T0C

            def w1w(k, m):
                c = k * NF1 + m * P
                return wpsb[:, c:c + P]

            def w2w(m):
                c = m * NO
                return wpsb2[:, c:c + NO]

            nc.scalar.dma_start(out=wpsb[:, 0:512], in_=wp[:, 0:512])
            nc.scalar.dma_start(out=wpsb[:, 512:1024], in_=wp[:, 512:1024])
            nc.scalar.dma_start(out=wpsb[:, 1024:W1T0C + P],
                                in_=wp[:, 1024:W1T0C + P])
            nc.scalar.dma_start(out=wpsb2, in_=wp2)
            b1sb = singles.tile([P, 2], f32)
            nc.scalar.dma_start(out=b1sb, in_=b1)
            b2sb = singles.tile([NO, 1], f32)
            nc.scalar.dma_start(out=b2sb, in_=b2)

            # ---- x split across BOTH hardware DGE rings (sync + scalar):
            # one ring alone saturates at ~95-200 GB/s, below the ~266 GB/s
            # the PE needs at roofline. Group 0 loads k-chunk by k-chunk on
            # sync (65KB each, so its matmuls can start almost immediately),
            # then odd groups ride sync and even groups ride scalar (which
            # first carries the weight prologue).
            xg0sb = singles.tile([P, KM, NB], f8)
            for k in range(KM):
                nc.sync.dma_start(out=xg0sb[:, k],
                                  in_=xb[0][:, k * NB:(k + 1) * NB])
            xtiles = [None] * NGRP
            xttiles = [None] * NBLK

            def load_group(g):
                xg = xp.tile([P, KM, NB], f8, tag="x", name=f"x_{g}")
                # g2 AND g3 ride scalar so neither queues behind the
                # ~790KB of g0+g1 on sync during the cold start
                ring = nc.scalar if g in (2, 3) or g % 2 == 0 else nc.sync
                ring.dma_start(out=xg, in_=xb[g])
                xtiles[g] = xg

            def load_tail(b):
                # tails load in pairs (one dma_start covers two blocks) on
                # the sync ring, which has slack during the cold start;
                # xtp has enough bufs that these never block on a pool wait
                if xttiles[b] is not None:
                    return
                n = 2 if b + 1 < NBLK else 1
                xt = xtp.tile([P, n, NB], f8, tag="xt", name=f"xt_{b}")
                nc.sync.dma_start(out=xt, in_=xtl[:, b:b + n])
                for i in range(n):
                    xttiles[b + i] = xt[:, i]

            load_group(1)
            load_group(2)
            load_group(3)
            load_tail(0)

            osbs = [None] * (NGRP // 4)

            def layer2(hs, g):
                # output staging is grouped 4 groups per buffer so interior
                # stores are fewer, bigger SWDGE transfers
                sb, slot = g // 4, g % 4
                if osbs[sb] is None:
                    osbs[sb] = op.tile([NO, 4, NB], f32, tag="o",
                                       name=f"o_{sb}")
                osb = osbs[sb]
                ps2 = ps2p.tile([NO, NB], f32, tag="ps2", name=f"ps2_{g}")
                for m in range(2):
                    nc.tensor.matmul(ps2, w2w(m), hs[m],
                                     start=(m == 0), stop=(m == 1))
                # bias on vector, relus on scalar: an engine that runs both
                # forms a cross-block dependency cycle (bias waits on L2
                # matmuls which wait on relus queued behind the bias)
                nc.vector.tensor_scalar_add(osb[:, slot], ps2, b2sb)
                if g >= NGRP - 2:
                    # final groups: short store chains, one per HW ring
                    gs = slice(g * NB, (g + 1) * NB)
                    ring = nc.scalar if g == NGRP - 2 else nc.sync
                    ring.dma_start(out=out[:, gs], in_=osb[:, slot])
                elif g == NGRP - 3:
                    gs = slice(12 * NB, 14 * NB)
                    nc.gpsimd.dma_start(out=out[:, gs], in_=osb[:, 0:2])
                elif slot == 3:
                    # interior stores ride the software DGE on the idle
                    # gpsimd queue (one 80KB store per ~7us keeps up)
                    gs = slice(sb * 4 * NB, (sb + 1) * 4 * NB)
                    nc.gpsimd.dma_start(out=out[:, gs], in_=osb)

            def relu_pair(g, pss):
                # relus split across scalar AND vector so the 4-relu chain
                # per block halves (PSUM banks recycle sooner). h carries
                # the x16 weight-transport scale (b1 is pre-scaled on the
                # host, W2 is pre-divided by 16), so the vector half is a
                # plain 2-op (add, max) tensor_scalar.
                hs = []
                for m in range(2):
                    h = hp.tile([P, NB], bf16, tag="h", name=f"h_{g}_{m}")
                    if m == 0:
                        nc.scalar.activation(h, pss[(g, m)], Relu,
                                             bias=b1sb[:, m:m + 1])
                    else:
                        nc.vector.tensor_scalar(h, pss[(g, m)],
                                                b1sb[:, m:m + 1], 0.0,
                                                add, amax)
                    hs.append(h)
                return hs

            def pack_pair(g, pss):
                # 16-row contraction tail for one group: 2 row-tiled
                # matmuls in distinct 32-row PE groups (concurrent)
                xt = xttiles[g // 2]
                base = 0 if g % 2 == 0 else 64
                for m in range(2):
                    rs = slice(base + 32 * m, base + 32 * m + KT)
                    nc.tensor.matmul(pss[(g, m)], wpsb[rs, W1T0:W1T0 + P],
                                     xt[rs], start=False, stop=True,
                                     tile_position=(base + 32 * m, 0))

            def pack_quad(blk, pss):
                g0, g1 = 2 * blk, 2 * blk + 1
                xt = xttiles[blk]
                for r, (g, m) in enumerate(
                        [(g0, 0), (g0, 1), (g1, 0), (g1, 1)]):
                    rs = slice(32 * r, 32 * r + KT)
                    nc.tensor.matmul(pss[(g, m)], wpsb[rs, W1T0:W1T0 + P],
                                     xt[rs], start=False, stop=True,
                                     tile_position=(32 * r, 0))

            pend = []
            # ================= block 0 (groups 0, 1) =================
            pss = {(g, m): ps1p.tile([P, NB], f32, tag="ps1",
                                     name=f"ps1_{g}_{m}")
                   for g in (0, 1) for m in range(2)}

            def mm0k(k):
                for m in range(2):
                    nc.tensor.matmul(pss[(0, m)], w1w(k, m), xg0sb[:, k],
                                     start=(k == 0), stop=False)

            # matmuls ordered by DMA arrival (weights wA=k0k1/wB=k2k3/wC
            # on scalar; x chunks k0..k5 on sync); fillers bridge the known
            # arrival bubbles so the HAM clock gate never sees a PE-idle
            # window
            mm0k(0)
            mm0k(1)                       # wA
            filler(6)
            mm0k(2)
            mm0k(3)                       # wB
            filler(6)
            mm0k(4)
            mm0k(5)                       # wC
            filler(4)
            # group 1 whole (N=512)
            for m in range(2):
                for k in range(KM):
                    nc.tensor.matmul(pss[(1, m)], w1w(k, m),
                                     xtiles[1][:, k],
                                     start=(k == 0), stop=False)
            load_group(4)
            load_group(5)
            load_tail(2)
            pack_quad(0, pss)
            pend.append((relu_pair(0, pss), 0))
            pend.append((relu_pair(1, pss), 1))

            # ================= blocks 1..7 =================
            for blk in range(1, NBLK):
                g0, g1 = 2 * blk, 2 * blk + 1
                if 2 * blk + 4 < NGRP:
                    load_group(2 * blk + 4)
                    load_group(2 * blk + 5)
                    if blk + 2 < NBLK:
                        load_tail(blk + 2)

                pss = {(g, m): ps1p.tile([P, NB], f32, tag="ps1",
                                         name=f"ps1_{g}_{m}")
                       for g in (g0, g1) for m in range(2)}
                primed = set()

                def kloop(g):
                    for m in range(2):
                        for k in range(KM):
                            nc.tensor.matmul(
                                pss[(g, m)], w1w(k, m), xtiles[g][:, k],
                                start=(k == 0 and (g, m) not in primed),
                                stop=False)

                if blk < NBLK - 1:
                    # pack right after the k-loops so the relus start as
                    # early as possible; the deferred layer-2 flushes run
                    # after (their h inputs are then long ready)
                    kloop(g0)
                    kloop(g1)
                    pack_quad(blk, pss)
                    while pend:
                        layer2(*pend.pop(0))
                    pend.append((relu_pair(g0, pss), g0))
                    pend.append((relu_pair(g1, pss), g1))
                else:
                    # last block: per-group tails so the final store chain
                    # is short. Both pending blocks' layer-2s flush BEFORE
                    # g15's k-loop so their vector bias-adds overlap PE
                    # work instead of serializing at the end, and g15's
                    # second relu runs on the (idle) gpsimd engine so the
                    # closing chain is one relu + one L2 + one bias + store.
                    kloop(g0)
                    pack_pair(g0, pss)
                    while pend:
                        layer2(*pend.pop(0))
                    hs14 = relu_pair(g0, pss)
                    kloop(g1)
                    pack_pair(g1, pss)
                    # g15's relus issue first (split scalar/vector) so they
                    # overlap g14's L2 on the PE; g15's bias rides scalar
                    h0 = hp.tile([P, NB], bf16, tag="h", name=f"h_{g1}_0")
                    nc.scalar.activation(h0, pss[(g1, 0)], Relu,
                                         bias=b1sb[:, 0:1])
                    h1 = hp.tile([P, NB], bf16, tag="h", name=f"h_{g1}_1")
                    nc.vector.tensor_scalar(h1, pss[(g1, 1)], b1sb[:, 1:2],
                                            0.0, add, amax)
                    layer2(hs14, g0)
                    # g15's bias-add on the scalar engine (free once h0 is
                    # done) so it doesn't queue behind h1's relu on vector
                    blk15, j15 = g1 // 4, g1 % 4
                    osb15 = osbs[blk15]
                    ps2f = ps2p.tile([NO, NB], f32, tag="ps2",
                                     name=f"ps2_{g1}")
                    for m in range(2):
                        nc.tensor.matmul(ps2f, w2w(m), [h0, h1][m],
                                         start=(m == 0), stop=(m == 1))
                    nc.scalar.activation(osb15[:, j15], ps2f, Identity,
                                         bias=b2sb)
                    gs = slice(g1 * NB, (g1 + 1) * NB)
                    nc.sync.dma_start(out=out[:, gs], in_=osb15[:, j15])

    nc.compile()
    return nc


def _fold_weights(conv_w, W1):
    """W1eff[784,256] such that x @ W1eff == flatten(conv(x)) @ W1.T."""
    cw = conv_w.astype(np.float64)
    W1r = W1.astype(np.float64).reshape(NF1, 26, 26).transpose(1, 2, 0)
    W1eff = np.zeros((28, 28, NF1), np.float64)
    for dr in range(3):
        for dc in range(3):
            W1eff[dr:dr + 26, dc:dc + 26, :] += cw[dr, dc] * W1r
    return W1eff.reshape(784, NF1)


def _prep_inputs(x, conv_w, W1, b1, W2, b2):
    bf16 = ml_dtypes.bfloat16
    W1eff = _fold_weights(conv_w, W1) * WSCALE
    wpack = np.zeros((P, W1T0C + P), np.float64)
    wpack[:, :KM * NF1] = W1eff[:768].reshape(KM, P, NF1).transpose(
        1, 0, 2).reshape(P, KM * NF1)
    W1T0 = KM * NF1
    wpack[0:KT, W1T0:W1T0 + P] = W1eff[768:784, 0:128]
    wpack[32:32 + KT, W1T0:W1T0 + P] = W1eff[768:784, 128:256]
    wpack[64:64 + KT, W1T0:W1T0 + P] = W1eff[768:784, 0:128]
    wpack[96:96 + KT, W1T0:W1T0 + P] = W1eff[768:784, 128:256]
    wpack = wpack.astype(np.float32).astype(f8)
    # h carries the x16 transport scale out of the relu, so W2 is divided
    # by 16 (exact, power of two) and b1 multiplied by 16 on the host
    wpack2 = np.ascontiguousarray((W2.T / WSCALE).astype(np.float64).reshape(
        2, P, NO).transpose(1, 0, 2).reshape(P, 2 * NO)).astype(bf16)
    b1p = np.ascontiguousarray(
        (b1 * WSCALE).astype(np.float32).reshape(2, P).T)
    b2p = b2.astype(np.float32).reshape(NO, 1)

    in_maps = []
    for c in range(NCORES):
        xcT = np.ascontiguousarray(
            x[c * BC:(c + 1) * BC].T).astype(f8)  # [784, BC]
        xmain = np.ascontiguousarray(
            xcT[:768].reshape(KM, P, NGRP, NB).transpose(2, 1, 0, 3)
        ).reshape(NGRP, P, KM * NB)
        xtail = np.zeros((P, NBLK, NB), f8)
        tl = xcT[768:784].reshape(KT, NBLK, 2, NB)
        xtail[0:KT] = tl[:, :, 0]
        xtail[32:32 + KT] = tl[:, :, 0]
        xtail[64:64 + KT] = tl[:, :, 1]
        xtail[96:96 + KT] = tl[:, :, 1]
        in_maps.append({
            "xb": xmain, "xtl": xtail,
            "wp": wpack, "wp2": wpack2, "b1": b1p, "b2": b2p,
        })
    return in_maps


def kernel(x, conv_w, W1, b1, W2, b2, _trace=False, _trace_kwargs=None):
    global _PROG
    from concourse import bass_utils

    x = np.asarray(x, dtype=np.float32)
    conv_w = np.asarray(conv_w, dtype=np.float32)
    W1 = np.asarray(W1, dtype=np.float32)
    b1 = np.asarray(b1, dtype=np.float32)
    W2 = np.asarray(W2, dtype=np.float32)
    b2 = np.asarray(b2, dtype=np.float32)
    assert x.shape == (B, 784), x.shape

    if _PROG is None:
        _PROG = _build_program()

    in_maps = _prep_inputs(x, conv_w, W1, b1, W2, b2)
    kwargs = dict(_trace_kwargs or {})
    res = bass_utils.run_bass_kernel_spmd(
        _PROG, in_maps, core_ids=list(range(NCORES)), trace=_trace, **kwargs)

    out = np.empty((B, NO), np.float32)
    for c in range(NCORES):
        out[c * BC:(c + 1) * BC] = res.results[c]["out"].T
    if _trace:
        return out, res
    return out

